# revision 10
# baseline (speedup 1.0000x reference)
"""Trainium2 Bass kernel for MinCutNet (dense mincut pooling GNN).

Sharding: data-parallel over graphs — 64 graphs per NeuronCore x 8 cores.
Device computes, per graph: x-projections, exact dense adjacency build
(DVE is_equal one-hots consumed by PE matmuls, PSUM-accumulated), both
message-passing products A@[u|1|h] and A@s, row softmax, and all pooling
bilinears, emitting a [112,17] graph-summary block. Host does only the
final [16,16]-scale per-graph tail (conv2 + MLP + losses; ~0.1% of FLOPs)
and the cross-shard mean of the two scalar losses.
"""
import numpy as np
import ml_dtypes

import concourse.bass as bass
import concourse.mybir as mybir
from concourse.tile import TileContext
from concourse.bass_utils import run_bass_kernel_spmd

BG, N, E, FIN, H, K, FOUT = 512, 256, 8192, 128, 32, 16, 10
NCORES = 8
G = BG // NCORES          # graphs per core
CH = E // 128             # 64 chunks of 128 edges per graph
BF16 = mybir.dt.bfloat16
LAST_EXEC_NS = None
F32 = mybir.dt.float32


def _split_tail_waits(nc, max_waits=1):
    # walrus codegen in this container rejects instructions with >1 sync
    # waits in several lowerings; move excess waits onto preceding NoOps on
    # the same engine (engine program order preserves the dependency).
    for fn in nc.m.functions:
        for bb in fn.blocks:
            new_insts = []
            for inst in bb.instructions:
                si = getattr(inst, 'sync_info', None)
                if si is not None and si.on_wait is not None and len(si.on_wait) > max_waits:
                    waits = list(si.on_wait)
                    keep = waits[-max_waits:]
                    extra = waits[:-max_waits]
                    for j in range(0, len(extra), max_waits):
                        new_insts.append(mybir.InstNoOp(
                            name=f"{inst.name}_sw{j}", engine=inst.engine,
                            ins=[], outs=[],
                            sync_info=mybir.SyncInfo(on_wait=extra[j:j+max_waits],
                                                     on_update=[])))
                    si.on_wait = keep
                new_insts.append(inst)
            bb.instructions = new_insts


def _build_program():
    nc = bass.Bass("TRN2", target_bir_lowering=False, debug=False)
    x_in = nc.dram_tensor("x", [G * N, FIN], F32, kind="ExternalInput")
    esrc_in = nc.dram_tensor("esrc", [128, G * CH], BF16, kind="ExternalInput")
    edst_in = nc.dram_tensor("edst", [128, G * CH], BF16, kind="ExternalInput")
    iota_in = nc.dram_tensor("iota", [128, N], BF16, kind="ExternalInput")
    ident_in = nc.dram_tensor("ident", [128, 128], F32, kind="ExternalInput")
    wall_in = nc.dram_tensor("wall", [128, 64], F32, kind="ExternalInput")
    cgm_in = nc.dram_tensor("cgm", [128, K], F32, kind="ExternalInput")
    crbm_in = nc.dram_tensor("crbm", [128, K], F32, kind="ExternalInput")
    blin1_in = nc.dram_tensor("blin1", [128, H], F32, kind="ExternalInput")
    bout = nc.dram_tensor("bout", [G, 112, 17], F32, kind="ExternalOutput")
    import os
    dbg = os.environ.get("K_DEBUG_A")
    aout = nc.dram_tensor("aout", [G, 128, 512], F32, kind="ExternalOutput") if dbg else None

    with TileContext(nc) as tc:
        with (
            tc.tile_pool(name="const", bufs=1) as cp,
            tc.tile_pool(name="edges", bufs=1) as ep,
            tc.tile_pool(name="work", bufs=3) as wp,
            tc.tile_pool(name="half", bufs=4) as hp,
            tc.tile_pool(name="psA", bufs=2, space="PSUM") as psA,
            tc.tile_pool(name="psS", bufs=1, space="PSUM") as psS,
            tc.tile_pool(name="psM", bufs=2, space="PSUM") as psM,
        ):
            iota = cp.tile([128, N], BF16, tag="iota")
            ident = cp.tile([128, 128], F32, tag="ident")
            wall = cp.tile([128, 64], F32, tag="wall")
            cgm = cp.tile([128, K], F32, tag="cgm")
            crbm = cp.tile([128, K], F32, tag="crbm")
            blin1 = cp.tile([128, H], F32, tag="blin1")
            nc.sync.dma_start(out=iota[:], in_=iota_in[:])
            nc.sync.dma_start(out=ident[:], in_=ident_in[:])
            nc.sync.dma_start(out=wall[:], in_=wall_in[:])
            nc.sync.dma_start(out=cgm[:], in_=cgm_in[:])
            nc.sync.dma_start(out=crbm[:], in_=crbm_in[:])
            nc.sync.dma_start(out=blin1[:], in_=blin1_in[:])
            esrc = ep.tile([128, G * CH], BF16, tag="esrc")
            edst = ep.tile([128, G * CH], BF16, tag="edst")
            nc.sync.dma_start(out=esrc[:], in_=esrc_in[:])
            nc.sync.dma_start(out=edst[:], in_=edst_in[:])

            for g in range(G):
                # ---- x load + transpose + projections -------------------
                xt = wp.tile([128, N], F32, tag="xt")
                for nh in range(2):
                    xf = wp.tile([128, FIN], F32, tag="xf")
                    nc.sync.dma_start(
                        out=xf[:], in_=x_in[g * N + nh * 128: g * N + (nh + 1) * 128, :])
                    xtp = psS.tile([128, 128], F32, tag="xtps", space="PSUM")
                    nc.tensor.transpose(out=xtp[:], in_=xf[:], identity=ident[:])
                    nc.scalar.copy(out=xt[:, nh * 128:(nh + 1) * 128], in_=xtp[:])

                # ---- exact adjacency via one-hot matmuls ----------------
                # A'[dst, src]: psum [128, 512] = [mh0 cols | mh1 cols]
                aps0 = psA.tile([128, N], F32, name="aps0", tag="aps", space="PSUM")
                aps1 = psA.tile([128, N], F32, name="aps1", tag="aps", space="PSUM")
                apsl = [aps0, aps1]
                for c in range(CH):
                    col = g * CH + c
                    soh = wp.tile([128, N], BF16, tag="soh")
                    doh = wp.tile([128, N], BF16, tag="doh")
                    nc.vector.tensor_tensor(
                        out=soh[:], in0=esrc[:, col:col + 1].to_broadcast([128, N]),
                        in1=iota[:], op=mybir.AluOpType.is_equal)
                    nc.vector.tensor_tensor(
                        out=doh[:], in0=edst[:, col:col + 1].to_broadcast([128, N]),
                        in1=iota[:], op=mybir.AluOpType.is_equal)
                    for mh in range(2):
                        nc.tensor.matmul(
                            out=apsl[mh][:],
                            lhsT=doh[:, mh * 128:(mh + 1) * 128], rhs=soh[:],
                            start=(c == 0), stop=(c == CH - 1))
                asb = wp.tile([128, 2 * N], F32, tag="asb")
                nc.scalar.copy(out=asb[:, 0:N], in_=aps0[:])
                nc.scalar.copy(out=asb[:, N:2 * N], in_=aps1[:])
                if aout is not None:
                    nc.sync.dma_start(out=aout[g], in_=asb[:])

                # ---- node-major projections: u|w|h ----------------------
                rhs1 = [hp.tile([128, 49], F32, name=f"rhs1_{nh}", tag=f"rhs1_{nh}") for nh in range(2)]
                wsb = [hp.tile([128, K], F32, name=f"wsb_{nh}", tag=f"wsb_{nh}") for nh in range(2)]
                Yt = [hp.tile([128, 112], F32, name=f"Y_{nh}", tag=f"Y_{nh}") for nh in range(2)]
                sext = [hp.tile([128, 17], F32, name=f"sext_{nh}", tag=f"sext_{nh}") for nh in range(2)]
                for nh in range(2):
                    uwh = psS.tile([128, 64], F32, tag="uwh", space="PSUM")
                    nc.tensor.matmul(out=uwh[:],
                                     lhsT=xt[:, nh * 128:(nh + 1) * 128],
                                     rhs=wall[:],
                                     start=True, stop=True)
                    nc.vector.tensor_copy(out=rhs1[nh][:, 0:16], in_=uwh[:, 0:16])
                    nc.vector.memset(rhs1[nh][:, 16:17], 1.0)
                    nc.vector.tensor_copy(out=wsb[nh][:], in_=uwh[:, 16:32])
                    nc.vector.tensor_add(out=rhs1[nh][:, 17:49], in0=uwh[:, 32:64],
                                         in1=blin1[:])
                    nc.vector.tensor_copy(out=Yt[nh][:, 0:32], in_=rhs1[nh][:, 17:49])
                    nc.vector.memset(sext[nh][:, 16:17], 1.0)

                # ---- pass1: m1 = A @ [u|1|h] ----------------------------
                for nh in range(2):
                    m1 = psM.tile([128, 49], F32, tag="m1", space="PSUM")
                    for mh in range(2):
                        nc.tensor.matmul(
                            out=m1[:],
                            lhsT=asb[:, mh * N + nh * 128: mh * N + (nh + 1) * 128],
                            rhs=rhs1[mh][:],
                            start=(mh == 0), stop=(mh == 1))
                    # s_logits = m1[:,0:16] + d*cg + w + crb
                    sl = hp.tile([128, K], F32, tag=f"sl_{nh}")
                    t2 = hp.tile([128, K], F32, tag=f"t2_{nh}")
                    nc.vector.tensor_add(out=sl[:], in0=m1[:, 0:16], in1=wsb[nh][:])
                    nc.vector.tensor_mul(
                        out=t2[:], in0=m1[:, 16:17].to_broadcast([128, K]), in1=cgm[:])
                    nc.vector.tensor_add(out=sl[:], in0=sl[:], in1=t2[:])
                    nc.vector.tensor_add(out=sl[:], in0=sl[:], in1=crbm[:])
                    # softmax
                    mx = hp.tile([128, 1], F32, tag=f"mx_{nh}")
                    nc.vector.tensor_reduce(out=mx[:], in_=sl[:],
                                            axis=mybir.AxisListType.X,
                                            op=mybir.AluOpType.max)
                    nc.vector.tensor_tensor(out=sl[:], in0=sl[:],
                                            in1=mx[:].to_broadcast([128, K]),
                                            op=mybir.AluOpType.subtract)
                    es = hp.tile([128, K], F32, tag=f"es_{nh}")
                    nc.scalar.activation(out=es[:], in_=sl[:],
                                         func=mybir.ActivationFunctionType.Exp)
                    sm = hp.tile([128, 1], F32, tag=f"sm_{nh}")
                    nc.vector.tensor_reduce(out=sm[:], in_=es[:],
                                            axis=mybir.AxisListType.X,
                                            op=mybir.AluOpType.add)
                    rc = hp.tile([128, 1], F32, tag=f"rc_{nh}")
                    nc.vector.reciprocal(out=rc[:], in_=sm[:])
                    nc.vector.tensor_tensor(out=sext[nh][:, 0:16], in0=es[:],
                                            in1=rc[:].to_broadcast([128, K]),
                                            op=mybir.AluOpType.mult)
                    nc.vector.tensor_copy(out=Yt[nh][:, 64:80], in_=sext[nh][:, 0:16])
                    # ds = d * s
                    nc.vector.tensor_tensor(out=Yt[nh][:, 96:112],
                                            in0=m1[:, 16:17].to_broadcast([128, K]),
                                            in1=sext[nh][:, 0:16],
                                            op=mybir.AluOpType.mult)
                    # Ah
                    nc.vector.tensor_copy(out=Yt[nh][:, 32:64], in_=m1[:, 17:49])

                # ---- A @ s ---------------------------------------------
                for nh in range(2):
                    asps = psS.tile([128, K], F32, tag="asps", space="PSUM")
                    for mh in range(2):
                        nc.tensor.matmul(
                            out=asps[:],
                            lhsT=asb[:, mh * N + nh * 128: mh * N + (nh + 1) * 128],
                            rhs=sext[mh][:, 0:16],
                            start=(mh == 0), stop=(mh == 1))
                    nc.vector.tensor_copy(out=Yt[nh][:, 80:96], in_=asps[:])

                # ---- bilinear block B = Y^T @ [s|1] ---------------------
                bps = psS.tile([112, 17], F32, tag="bps", space="PSUM")
                for nh in range(2):
                    nc.tensor.matmul(out=bps[:], lhsT=Yt[nh][:],
                                     rhs=sext[nh][:],
                                     start=(nh == 0), stop=(nh == 1))
                bsb = wp.tile([112, 17], F32, tag="bsb")
                nc.scalar.copy(out=bsb[:], in_=bps[:])
                nc.sync.dma_start(out=bout[g], in_=bsb[:])

    _split_tail_waits(nc)
    return nc


def kernel(x, edge_index, batch, W_lin1, b_lin1, W_rel1, b_rel1, W_root1,
           W_pool, b_pool, W_rel2, b_rel2, W_root2, W_lin2, b_lin2,
           W_lin3, b_lin3):
    x = np.asarray(x, np.float32)
    src = np.asarray(edge_index[0], np.int64)
    dst = np.asarray(edge_index[1], np.int64)
    for nm in ['W_lin1', 'b_lin1', 'W_rel1', 'b_rel1', 'W_root1', 'W_pool',
               'b_pool', 'W_rel2', 'b_rel2', 'W_root2', 'W_lin2', 'b_lin2',
               'W_lin3', 'b_lin3']:
        locals()
    W_lin1 = np.asarray(W_lin1, np.float32); b_lin1 = np.asarray(b_lin1, np.float32)
    W_rel1 = np.asarray(W_rel1, np.float32); b_rel1 = np.asarray(b_rel1, np.float32)
    W_root1 = np.asarray(W_root1, np.float32)
    W_pool = np.asarray(W_pool, np.float32); b_pool = np.asarray(b_pool, np.float32)
    W_rel2 = np.asarray(W_rel2, np.float32); b_rel2 = np.asarray(b_rel2, np.float32)
    W_root2 = np.asarray(W_root2, np.float32)
    W_lin2 = np.asarray(W_lin2, np.float32); b_lin2 = np.asarray(b_lin2, np.float32)
    W_lin3 = np.asarray(W_lin3, np.float32); b_lin3 = np.asarray(b_lin3, np.float32)

    # host-fused weight products (weights-only, [128,16]-scale)
    Wg = W_lin1 @ W_rel1 @ W_pool
    Wr = W_lin1 @ W_root1 @ W_pool
    cg = b_lin1 @ W_rel1 @ W_pool
    crb = b_lin1 @ W_root1 @ W_pool + b_pool

    bf = ml_dtypes.bfloat16
    wall = np.concatenate([Wg, Wr, W_lin1], axis=1).astype(np.float32)  # [128,64]
    iota = np.broadcast_to(np.arange(N, dtype=np.float32), (128, N)).astype(bf)
    ident = np.eye(128, dtype=np.float32)
    cgm = np.broadcast_to(cg, (128, K)).astype(np.float32).copy()
    crbm = np.broadcast_to(crb, (128, K)).astype(np.float32).copy()
    blin1 = np.broadcast_to(b_lin1, (128, H)).astype(np.float32).copy()

    # local edge coordinates (layout prep for graph-sharding)
    src_l = (src % N).astype(np.float32).astype(bf)
    dst_l = (dst % N).astype(np.float32).astype(bf)

    nc = _build_program()
    in_maps = []
    for core in range(NCORES):
        g0 = core * G
        sl = slice(g0 * E, (g0 + G) * E)
        esrc = src_l[sl].reshape(G, 128, CH).transpose(1, 0, 2).reshape(128, G * CH)
        edst = dst_l[sl].reshape(G, 128, CH).transpose(1, 0, 2).reshape(128, G * CH)
        in_maps.append({
            "x": np.ascontiguousarray(x[g0 * N:(g0 + G) * N]),
            "esrc": np.ascontiguousarray(esrc),
            "edst": np.ascontiguousarray(edst),
            "iota": iota, "ident": ident, "wall": wall,
            "cgm": cgm, "crbm": crbm, "blin1": blin1,
        })
    import os as _os, time as _time
    res = run_bass_kernel_spmd(nc, in_maps, list(range(NCORES)))
    global LAST_EXEC_NS
    LAST_EXEC_NS = res.exec_time_ns
    if _os.environ.get("BASS_KERNEL_TIME"):
        t0 = _time.time()
        res = run_bass_kernel_spmd(nc, in_maps, list(range(NCORES)))
        LAST_EXEC_NS = int((_time.time() - t0) * 1e9)

    # ---- host tail: [16,16]-scale per-graph finale ---------------------
    eyeK = np.eye(K, dtype=np.float32)
    logits = np.zeros((BG, FOUT), np.float32)
    mcs = np.zeros(BG, np.float32)
    ors = np.zeros(BG, np.float32)
    for core in range(NCORES):
        Bb = np.asarray(res.results[core]["bout"], np.float32)  # [G,112,17]
        for gl in range(G):
            g = core * G + gl
            Bblk = Bb[gl]
            hTs = Bblk[0:32, 0:16]; AhTs = Bblk[32:64, 0:16]
            ss = Bblk[64:80, 0:16]; AsTs = Bblk[80:96, 0:16]
            dsTs = Bblk[96:112, 0:16]
            S1 = Bblk[64:80, 16]
            out1 = AhTs.T @ W_rel1 + S1[:, None] * b_rel1[None, :] + hTs.T @ W_root1
            out_adj = AsTs.T
            num = np.trace(out_adj); den = np.trace(dsTs)
            mcs[g] = -(num / den)
            ssn = np.linalg.norm(ss)
            ors[g] = np.linalg.norm(ss / ssn - eyeK / np.sqrt(np.float32(K)))
            oa2 = out_adj * (1.0 - eyeK)
            dd = np.sqrt(oa2.sum(-1))[None, :] + 1e-15
            oan = oa2 / dd / dd.T
            h3 = (oan @ out1) @ W_rel2 + b_rel2 + out1 @ W_root2
            r = h3.sum(0)
            r2 = np.maximum(r @ W_lin2 + b_lin2, 0.0)
            lg = r2 @ W_lin3 + b_lin3
            m = lg.max()
            logits[g] = lg - (np.log(np.exp(lg - m).sum()) + m)
    return (logits, np.float32(mcs.mean()), np.float32(ors.mean()))
